# revision 1
# baseline (speedup 1.0000x reference)
"""Single-head causal self-attention on 8 NeuronCores (data-parallel over batch).

Reference computation (per batch element b):
    Q = X @ Wq + bq; K = X @ Wk + bk; V = X @ Wv + bv        # [T, DK]
    S = Q @ K.T / sqrt(DK)  (causal masked)
    out = softmax(S) @ V                                      # [T, DK]

Device strategy (one batch element per core):
  - Host passes X.T [C, T] so every DMA row is contiguous.
  - Two full-width projection passes with packed stationaries:
      pass A: [Wv | Wk] -> psum rows 0:64 = V.T, rows 64:128 = K.T
      pass B: [Wq | Wq] -> Q.T duplicated in both partition halves
    Biases are added exactly during the PSUM->SBUF drain (tensor_scalar_add
    with a per-partition vector).
  - V.T is PE-transposed into natural [s, dk] tiles with a ones column
    appended; the ones column makes the output matmul also produce the
    softmax denominator l (row 64 of the output).
  - Scores are computed transposed: S.T[s, t] = K.T^T @ Q.T, so softmax's
    exp (ScalarE, scale=1/8 fused) and the P@V contraction (over s = the
    partition dim) need no large transposes. Causality = skip tiles below
    the diagonal + one upper-triangular 128x128 mask multiply per s-tile.
  - Device output per core: [65, T] = rows 0:64 unnormalized O.T, row 64 l.
    Host computes (O_unnorm / l).T.
"""

import sys

sys.path.insert(0, "/opt/trn_rl_repo")

import numpy as np

B, T, C, DK = 8, 2048, 1024, 64
KT = C // 128          # 8 k-tiles in the contraction over C
NS = T // 128          # 16 s-tiles (key blocks)
NCHUNK = T // 512      # 4 output chunks of 512
SCALE = 1.0 / np.sqrt(DK)

_CACHE = {}


def _build():
    from concourse import bass, bacc, tile

    mybir = bass.mybir
    f32 = mybir.dt.float32
    f32r = mybir.dt.float32r

    nc = bacc.Bacc(
        "TRN2", target_bir_lowering=False, debug=False, num_devices=B
    )

    xt_d = nc.dram_tensor("xt", [KT, 128, T], f32r, kind="ExternalInput")
    wvk_d = nc.dram_tensor("wvk", [128, KT * 128], f32r, kind="ExternalInput")
    wqq_d = nc.dram_tensor("wqq", [128, KT * 128], f32r, kind="ExternalInput")
    bvk_d = nc.dram_tensor("bvk", [128, 1], f32, kind="ExternalInput")
    bqq_d = nc.dram_tensor("bqq", [128, 1], f32, kind="ExternalInput")
    out_d = nc.dram_tensor("out", [65, T], f32, kind="ExternalOutput")

    # one packed const block: cols 0:128 tri-mask, 128:192 ident (rows 0:64),
    # col 192 reserved
    cst_np = np.zeros((128, 193), dtype=np.float32)
    cst_np[:, 0:128] = np.triu(np.ones((128, 128), dtype=np.float32))
    cst_np[0:64, 128:192] = np.eye(64, dtype=np.float32)
    cst_d = nc.inline_tensor(cst_np, "cst")

    EXP = mybir.ActivationFunctionType.Exp

    with tile.TileContext(nc) as tc:
        with tc.tile_pool(name="const", bufs=1) as cpool, \
             tc.tile_pool(name="weights", bufs=1) as wpool, \
             tc.tile_pool(name="x", bufs=1) as xpool, \
             tc.tile_pool(name="acts", bufs=1) as apool:

            cst = cpool.tile([128, 193], f32r)
            nc.gpsimd.dma_start(out=cst[:], in_=cst_d[:].bitcast(f32r))
            tri = cst[:, 0:128]
            ident = cst[0:64, 128:192]
            bvk = cpool.tile([128, 1], f32)
            nc.gpsimd.dma_start(out=bvk[:], in_=bvk_d[:])
            bqq = cpool.tile([128, 1], f32)
            nc.gpsimd.dma_start(out=bqq[:], in_=bqq_d[:])

            wvk = wpool.tile([128, KT * 128], f32r)
            wqq = wpool.tile([128, KT * 128], f32r)
            nc.scalar.dma_start(out=wvk[:], in_=wvk_d[:])
            nc.scalar.dma_start(out=wqq[:], in_=wqq_d[:])

            dma_engs = [nc.sync, nc.gpsimd, nc.scalar]
            xts = []
            for k in range(KT):
                xk = xpool.tile([128, T], f32r, tag=f"x{k}")
                dma_engs[k % 3].dma_start(out=xk[:], in_=xt_d[k])
                xts.append(xk)

            # persistent activations
            vk = apool.tile([128, T], f32r, tag="vk")    # V.T rows 0:64, K.T rows 64:128
            qq = apool.tile([128, T], f32r, tag="qq")    # Q.T in both halves
            v1 = apool.tile([128, NS * 65], f32r, tag="v1")  # [V_i | 1] stationaries
            osb = apool.tile([65, T], f32, tag="osb")

            nc.gpsimd.memset(v1[:].bitcast(f32), 1.0)

            # ---------------- projections ----------------
            with tc.tile_pool(name="pproj", bufs=1, space="PSUM") as pproj:
                psA = pproj.tile([128, T], f32, tag="psA")
                psB = pproj.tile([128, T], f32, tag="psB")
                for k in range(KT):
                    for c in range(NCHUNK):
                        sl = slice(512 * c, 512 * (c + 1))
                        nc.tensor.matmul(
                            psA[:, sl],
                            wvk[:, 128 * k:128 * (k + 1)],
                            xts[k][:, sl],
                            start=(k == 0), stop=(k == KT - 1),
                        )
                    for c in range(NCHUNK):
                        sl = slice(512 * c, 512 * (c + 1))
                        nc.tensor.matmul(
                            psB[:, sl],
                            wqq[:, 128 * k:128 * (k + 1)],
                            xts[k][:, sl],
                            start=(k == 0), stop=(k == KT - 1),
                        )
                # drain with exact bias add, pipelined per 512-chunk
                for c in range(NCHUNK):
                    sl = slice(512 * c, 512 * (c + 1))
                    nc.vector.tensor_scalar_add(vk[:, sl], psA[:, sl], bvk[:])
                    nc.vector.tensor_scalar_add(qq[:, sl], psB[:, sl], bqq[:])

            # ---------------- V transposes ----------------
            with tc.tile_pool(name="pv", bufs=2, space="PSUM") as pv:
                for i in range(NS):
                    vt = pv.tile([128, 64], f32r, tag="vt")
                    nc.tensor.transpose(
                        vt[:], vk[0:64, 128 * i:128 * (i + 1)], ident[:]
                    )
                    nc.vector.tensor_copy(v1[:, 65 * i:65 * i + 64], vt[:])

            # ---------------- attention ----------------
            with tc.tile_pool(name="po", bufs=1, space="PSUM") as po, \
                 tc.tile_pool(name="pst", bufs=2, space="PSUM") as pst, \
                 tc.tile_pool(name="et", bufs=3) as etpool:

                ops = [
                    po.tile([65, 512], f32, tag=f"o{j}", name=f"o{j}")
                    for j in range(NCHUNK)
                ]

                for i in range(NS):
                    ts = 128 * i
                    jmin = i // 4
                    et = etpool.tile([128, T], f32r, tag="et")
                    if ts > 512 * jmin:
                        nc.gpsimd.memset(et[:, 512 * jmin:ts].bitcast(f32), 0.0)
                    for tb in range(ts // 1024, 2):
                        st = pst.tile([128, 1024], f32, tag="st")
                        for cc in range(2):
                            t0 = 1024 * tb + 512 * cc
                            if t0 + 512 <= ts:
                                continue
                            nc.tensor.matmul(
                                st[:, 512 * cc:512 * (cc + 1)],
                                vk[64:128, 128 * i:128 * (i + 1)],
                                qq[64:128, t0:t0 + 512],
                                start=True, stop=True,
                            )
                        off = max(0, ts - 1024 * tb)
                        nc.scalar.activation(
                            et[:, 1024 * tb + off:1024 * (tb + 1)],
                            st[:, off:1024],
                            EXP, scale=SCALE,
                        )
                    # causal mask on the diagonal 128-block
                    nc.vector.tensor_mul(
                        et[:, ts:ts + 128], et[:, ts:ts + 128], tri[:]
                    )
                    for j in range(jmin, NCHUNK):
                        nc.tensor.matmul(
                            ops[j][:],
                            v1[:, 65 * i:65 * i + 65],
                            et[:, 512 * j:512 * (j + 1)],
                            start=(i == 0), stop=(i == 4 * j + 3),
                        )
                    # drain any output chunk whose accumulation just finished
                    for j in range(jmin, NCHUNK):
                        if i == 4 * j + 3:
                            sl = slice(512 * j, 512 * (j + 1))
                            nc.vector.tensor_copy(osb[:, sl], ops[j][:])
                            nc.sync.dma_start(out=out_d[:, sl], in_=osb[:, sl])

    nc.compile()
    return nc


def _get_nc():
    if "nc" not in _CACHE:
        _CACHE["nc"] = _build()
    return _CACHE["nc"]


def make_in_maps(X, Wq, bq, Wk, bk, Wv, bv):
    X = np.asarray(X, dtype=np.float32)
    Wq = np.asarray(Wq, dtype=np.float32)
    Wk = np.asarray(Wk, dtype=np.float32)
    Wv = np.asarray(Wv, dtype=np.float32)
    bq = np.asarray(bq, dtype=np.float32)
    bk = np.asarray(bk, dtype=np.float32)
    bv = np.asarray(bv, dtype=np.float32)

    wvk = np.ascontiguousarray(
        np.concatenate([Wv, Wk], axis=1).reshape(KT, 128, 128)
        .transpose(1, 0, 2).reshape(128, KT * 128)
    )
    wqq = np.ascontiguousarray(
        np.concatenate([Wq, Wq], axis=1).reshape(KT, 128, 128)
        .transpose(1, 0, 2).reshape(128, KT * 128)
    )
    bvk = np.concatenate([bv, bk]).reshape(128, 1).astype(np.float32)
    bqq = np.concatenate([bq, bq]).reshape(128, 1).astype(np.float32)

    in_maps = []
    for b in range(B):
        xt = np.ascontiguousarray(X[b].T).reshape(KT, 128, T)
        in_maps.append(
            {"xt": xt, "wvk": wvk, "wqq": wqq, "bvk": bvk, "bqq": bqq}
        )
    return in_maps


def kernel(X, Wq, bq, Wk, bk, Wv, bv):
    from concourse.bass_utils import run_bass_kernel_spmd

    nc = _get_nc()
    in_maps = make_in_maps(X, Wq, bq, Wk, bk, Wv, bv)
    res = run_bass_kernel_spmd(nc, in_maps, list(range(B)))

    out = np.empty((B, T, DK), dtype=np.float32)
    for b in range(B):
        r = res.results[b]["out"]
        out[b] = (r[:64] / r[64:65]).T
    return out



# revision 4
# speedup vs baseline: 1.2174x; 1.2174x over previous
"""Single-head causal self-attention on 8 NeuronCores (data-parallel over batch).

Reference computation (per batch element b):
    Q = X @ Wq + bq; K = X @ Wk + bk; V = X @ Wv + bv        # [T, DK]
    S = Q @ K.T / sqrt(DK)  (causal masked)
    out = softmax(S) @ V                                      # [T, DK]

Device strategy (one batch element per core), fp16 compute / fp32 accumulate:
  - Host passes X.T [C, T] in fp16 so every DMA row is contiguous and
    HBM+tunnel traffic is halved vs fp32.
  - Two full-width projection passes with packed fp16 stationaries:
      pass A: [Wv | Wk] -> psum rows 0:64 = V.T, rows 64:128 = K.T
      pass B: [Wq | Wq] -> Q.T duplicated in both partition halves
    Biases are added exactly during the PSUM->SBUF drain (tensor_scalar_add
    with a per-partition fp32 vector, fp16 output).
  - V.T is PE-transposed into natural [s, dk] tiles with a ones column
    appended; the ones column makes the output matmul also produce the
    softmax denominator l (row 64 of the output).
  - Scores are computed transposed: S.T[s, t] = K.T^T @ Q.T, so softmax's
    exp (ScalarE, scale=1/8 fused) and the P@V contraction (over s = the
    partition dim) need no large transposes.
  - Attention is streamed per 512-column output chunk j (j-outer): for
    each j, s-tiles i = 0..4j+3 produce a <=512-col score piece, exp'd to
    fp16, then accumulated into po[j].  Score/exp/PV work is trimmed to
    start exactly at the diagonal column ts = 128*i (no sub-diagonal
    columns, no memsets); only the diagonal 128-block needs a triangular
    mask multiply.  Small PSUM pieces (1 bank each) allow a deep rotation
    so the PE array can run 2 score pieces ahead of the PV consumer and
    never stalls on ScalarE's exp.
  - Device output per core: [65, T] = rows 0:64 unnormalized O.T, row 64 l.
    Host computes (O_unnorm / l).T.
"""

import sys

sys.path.insert(0, "/opt/trn_rl_repo")

import numpy as np

B, T, C, DK = 8, 2048, 1024, 64
KT = C // 128          # 8 k-tiles in the contraction over C
NS = T // 128          # 16 s-tiles (key blocks)
NCHUNK = T // 512      # 4 output chunks of 512
SCALE = 1.0 / np.sqrt(DK)

_CACHE = {}


def _build():
    from concourse import bass, bacc, tile

    mybir = bass.mybir
    f32 = mybir.dt.float32
    f16 = mybir.dt.float16

    nc = bacc.Bacc(
        "TRN2", target_bir_lowering=False, debug=False, num_devices=B
    )

    xt_d = nc.dram_tensor("xt", [KT, 128, T], f16, kind="ExternalInput")
    wvk_d = nc.dram_tensor("wvk", [128, KT * 128], f16, kind="ExternalInput")
    wqq_d = nc.dram_tensor("wqq", [128, KT * 128], f16, kind="ExternalInput")
    bvk_d = nc.dram_tensor("bvk", [128, 1], f32, kind="ExternalInput")
    bqq_d = nc.dram_tensor("bqq", [128, 1], f32, kind="ExternalInput")
    out_d = nc.dram_tensor("out", [65, T], f32, kind="ExternalOutput")

    # one packed const block: cols 0:128 tri-mask, 128:192 ident (rows 0:64)
    cst_np = np.zeros((128, 192), dtype=np.float16)
    cst_np[:, 0:128] = np.triu(np.ones((128, 128), dtype=np.float16))
    cst_np[0:64, 128:192] = np.eye(64, dtype=np.float16)
    cst_d = nc.inline_tensor(cst_np, "cst")

    EXP = mybir.ActivationFunctionType.Exp

    with tile.TileContext(nc) as tc:
        with tc.tile_pool(name="const", bufs=1) as cpool, \
             tc.tile_pool(name="weights", bufs=1) as wpool, \
             tc.tile_pool(name="x", bufs=1) as xpool, \
             tc.tile_pool(name="acts", bufs=1) as apool:

            # x tiles first on every queue so the PE can start ASAP
            dma_engs = [nc.sync, nc.gpsimd, nc.scalar]
            xts = []
            for k in range(KT):
                xk = xpool.tile([128, T], f16, tag=f"x{k}")
                dma_engs[k % 3].dma_start(out=xk[:], in_=xt_d[k])
                xts.append(xk)

            wvk = wpool.tile([128, KT * 128], f16)
            wqq = wpool.tile([128, KT * 128], f16)
            nc.sync.dma_start(out=wvk[:], in_=wvk_d[:])
            nc.gpsimd.dma_start(out=wqq[:], in_=wqq_d[:])

            cst = cpool.tile([128, 192], f16)
            nc.scalar.dma_start(out=cst[:], in_=cst_d[:])
            tri = cst[:, 0:128]
            ident = cst[0:64, 128:192]
            bvk = cpool.tile([128, 1], f32)
            nc.sync.dma_start(out=bvk[:], in_=bvk_d[:])
            bqq = cpool.tile([128, 1], f32)
            nc.gpsimd.dma_start(out=bqq[:], in_=bqq_d[:])

            # persistent activations
            vk = apool.tile([128, T], f16, tag="vk")    # V.T rows 0:64, K.T rows 64:128
            qq = apool.tile([128, T], f16, tag="qq")    # Q.T in both halves
            v1 = apool.tile([128, NS * 65], f16, tag="v1")  # [V_i | 1] stationaries
            osb = apool.tile([65, T], f32, tag="osb")

            nc.gpsimd.memset(v1[:], 1.0)

            # ---------------- projections ----------------
            with tc.tile_pool(name="pproj", bufs=1, space="PSUM") as pproj:
                psA = pproj.tile([128, T], f32, tag="psA")
                psB = pproj.tile([128, T], f32, tag="psB")
                for k in range(KT):
                    for c in range(NCHUNK):
                        sl = slice(512 * c, 512 * (c + 1))
                        nc.tensor.matmul(
                            psA[:, sl],
                            wvk[:, 128 * k:128 * (k + 1)],
                            xts[k][:, sl],
                            start=(k == 0), stop=(k == KT - 1),
                        )
                    for c in range(NCHUNK):
                        sl = slice(512 * c, 512 * (c + 1))
                        nc.tensor.matmul(
                            psB[:, sl],
                            wqq[:, 128 * k:128 * (k + 1)],
                            xts[k][:, sl],
                            start=(k == 0), stop=(k == KT - 1),
                        )
                # drain with exact bias add, pipelined per 512-chunk
                for c in range(NCHUNK):
                    sl = slice(512 * c, 512 * (c + 1))
                    nc.vector.tensor_scalar_add(vk[:, sl], psA[:, sl], bvk[:])
                    nc.vector.tensor_scalar_add(qq[:, sl], psB[:, sl], bqq[:])

            # ---------------- V transposes ----------------
            with tc.tile_pool(name="pv", bufs=2, space="PSUM") as pv:
                for i in range(NS):
                    vt = pv.tile([128, 64], f16, tag="vt")
                    nc.tensor.transpose(
                        vt[:], vk[0:64, 128 * i:128 * (i + 1)], ident[:]
                    )
                    nc.vector.tensor_copy(v1[:, 65 * i:65 * i + 64], vt[:])

            # ---------------- attention (j-outer streaming) ----------------
            # For output chunk j (cols [512j, 512j+512)), s-tiles i=0..4j+3:
            #   S piece: st = K_i.T^T @ Q.T[:, c0:512(j+1)],  c0 = max(512j, ts)
            #   exp:     et = exp(st/8) fp16 (ScalarE, fused scale)
            #   mask:    tri multiply when the diagonal block is in this chunk
            #   PV:      po[j][:, c0-512j:] += [V_i|1]^T @ et
            # Tensor-queue order runs 2 score pieces ahead of PV so the PE
            # never waits for exp.
            with tc.tile_pool(name="po", bufs=2, space="PSUM") as po, \
                 tc.tile_pool(name="pst", bufs=6, space="PSUM") as pst, \
                 tc.tile_pool(name="et", bufs=4) as etpool:

                pend = {}

                def s_piece(i, j, oj):
                    ts = 128 * i
                    c0 = max(512 * j, ts)
                    w = 512 * (j + 1) - c0
                    st = pst.tile([128, 512], f32, tag="st")
                    et = etpool.tile([128, 512], f16, tag="et")
                    nc.tensor.matmul(
                        st[:, 0:w],
                        vk[64:128, ts:ts + 128],
                        qq[64:128, c0:c0 + w],
                        start=True, stop=True,
                    )
                    nc.scalar.activation(et[:, 0:w], st[:, 0:w], EXP, scale=SCALE)
                    if ts >= 512 * j:  # diagonal block lives in this chunk
                        nc.vector.tensor_mul(
                            et[:, 0:128], et[:, 0:128], tri[:]
                        )
                    pend[(i, j)] = (et, c0, w, oj)

                def pv_piece(i, j):
                    et, c0, w, oj = pend.pop((i, j))
                    nc.tensor.matmul(
                        oj[:, c0 - 512 * j:c0 - 512 * j + w],
                        v1[:, 65 * i:65 * i + 65],
                        et[:, 0:w],
                        start=(i == 0), stop=(i == 4 * j + 3),
                    )

                for j in range(NCHUNK):
                    oj = po.tile([65, 512], f32, tag="o")
                    ni = 4 * j + 4          # s-tiles contributing to chunk j
                    LOOK = 2
                    for i in range(ni + LOOK):
                        if i < ni:
                            s_piece(i, j, oj)
                        if i >= LOOK:
                            pv_piece(i - LOOK, j)
                    # drain chunk j
                    sl = slice(512 * j, 512 * (j + 1))
                    nc.vector.tensor_copy(osb[:, sl], oj[:])
                    nc.sync.dma_start(out=out_d[:, sl], in_=osb[:, sl])

    nc.compile()
    return nc


def _get_nc():
    if "nc" not in _CACHE:
        _CACHE["nc"] = _build()
    return _CACHE["nc"]


def make_in_maps(X, Wq, bq, Wk, bk, Wv, bv):
    X = np.asarray(X, dtype=np.float32)
    Wq = np.asarray(Wq, dtype=np.float32)
    Wk = np.asarray(Wk, dtype=np.float32)
    Wv = np.asarray(Wv, dtype=np.float32)
    bq = np.asarray(bq, dtype=np.float32)
    bk = np.asarray(bk, dtype=np.float32)
    bv = np.asarray(bv, dtype=np.float32)

    wvk = np.ascontiguousarray(
        np.concatenate([Wv, Wk], axis=1).reshape(KT, 128, 128)
        .transpose(1, 0, 2).reshape(128, KT * 128)
    ).astype(np.float16)
    wqq = np.ascontiguousarray(
        np.concatenate([Wq, Wq], axis=1).reshape(KT, 128, 128)
        .transpose(1, 0, 2).reshape(128, KT * 128)
    ).astype(np.float16)
    bvk = np.concatenate([bv, bk]).reshape(128, 1).astype(np.float32)
    bqq = np.concatenate([bq, bq]).reshape(128, 1).astype(np.float32)

    in_maps = []
    for b in range(B):
        xt = (
            np.ascontiguousarray(X[b].T)
            .astype(np.float16)
            .reshape(KT, 128, T)
        )
        in_maps.append(
            {"xt": xt, "wvk": wvk, "wqq": wqq, "bvk": bvk, "bqq": bqq}
        )
    return in_maps


def kernel(X, Wq, bq, Wk, bk, Wv, bv):
    from concourse.bass_utils import run_bass_kernel_spmd

    nc = _get_nc()
    in_maps = make_in_maps(X, Wq, bq, Wk, bk, Wv, bv)
    res = run_bass_kernel_spmd(nc, in_maps, list(range(B)))

    out = np.empty((B, T, DK), dtype=np.float32)
    for b in range(B):
        r = res.results[b]["out"]
        out[b] = (r[:64] / r[64:65]).T
    return out


# revision 6
# speedup vs baseline: 1.2609x; 1.0358x over previous
"""Single-head causal self-attention on 8 NeuronCores (data-parallel over batch).

Reference computation (per batch element b):
    Q = X @ Wq + bq; K = X @ Wk + bk; V = X @ Wv + bv        # [T, DK]
    S = Q @ K.T / sqrt(DK)  (causal masked)
    out = softmax(S) @ V                                      # [T, DK]

Device strategy (one batch element per core), fp16 compute / fp32 accumulate:
  - Host passes X.T [C, T] in fp16 so every DMA row is contiguous and
    HBM+tunnel traffic is halved vs fp32.
  - DMA order puts the stationaries (wvk, wqq) and the first X tile at the
    head of the three DMA queues so the PE can start within ~5us of the
    queues opening instead of waiting behind 1.5MB of X.
  - Two full-width projection passes with packed fp16 stationaries:
      pass A: [Wv | Wk] -> psum rows 0:64 = V.T, rows 64:128 = K.T
      pass B: [Wq | Wq] -> Q.T duplicated in both partition halves
    Biases are added exactly during the PSUM->SBUF drain (tensor_scalar_add
    with a per-partition fp32 vector, fp16 output).
  - V.T is PE-transposed into natural [s, dk] tiles with a ones column
    appended; the ones column makes the output matmul also produce the
    softmax denominator l (row 64 of the output).
  - Scores are computed transposed: S.T[s, t] = K.T^T @ Q.T, so softmax's
    exp (ScalarE, scale=1/8 fused) and the P@V contraction (over s = the
    partition dim) need no large transposes.
  - Causality: score/exp/PV work is trimmed to start exactly at the
    diagonal column ts = 128*i (no sub-diagonal columns, no memsets);
    only the diagonal 128-block needs a triangular mask multiply.
  - Tensor-queue order runs one s-tile of score lookahead (S(i+1) issued
    before PV(i)) so the PE array does not stall waiting for exp(i).
  - Device output per core: [65, T] = rows 0:64 unnormalized O.T, row 64 l.
    Host computes (O_unnorm / l).T.
"""

import sys

sys.path.insert(0, "/opt/trn_rl_repo")

import numpy as np

B, T, C, DK = 8, 2048, 1024, 64
KT = C // 128          # 8 k-tiles in the contraction over C
NS = T // 128          # 16 s-tiles (key blocks)
NCHUNK = T // 512      # 4 output chunks of 512
SCALE = 1.0 / np.sqrt(DK)
PCHUNK = 512           # projection matmul column width

_CACHE = {}


def _build():
    from concourse import bass, bacc, tile

    mybir = bass.mybir
    f32 = mybir.dt.float32
    f16 = mybir.dt.float16

    nc = bacc.Bacc(
        "TRN2", target_bir_lowering=False, debug=False, num_devices=B
    )

    xt_d = nc.dram_tensor("xt", [KT, 128, T], f16, kind="ExternalInput")
    wvk_d = nc.dram_tensor("wvk", [128, KT * 128], f16, kind="ExternalInput")
    wqq_d = nc.dram_tensor("wqq", [128, KT * 128], f16, kind="ExternalInput")
    bvk_d = nc.dram_tensor("bvk", [128, 1], f32, kind="ExternalInput")
    bqq_d = nc.dram_tensor("bqq", [128, 1], f32, kind="ExternalInput")
    out_d = nc.dram_tensor("out", [65, T], f32, kind="ExternalOutput")

    # one packed const block: cols 0:128 tri-mask, 128:192 ident (rows 0:64)
    cst_np = np.zeros((128, 192), dtype=np.float16)
    cst_np[:, 0:128] = np.triu(np.ones((128, 128), dtype=np.float16))
    cst_np[0:64, 128:192] = np.eye(64, dtype=np.float16)
    cst_d = nc.inline_tensor(cst_np, "cst")

    EXP = mybir.ActivationFunctionType.Exp

    with tile.TileContext(nc) as tc:
        with tc.tile_pool(name="persist", bufs=1) as ppool, \
             tc.tile_pool(name="x", bufs=1) as xpool:

            # stationaries at the head of the queues, then x tiles
            # round-robin; x0 is split across sync+gpsimd so the first
            # projection matmul can start as early as possible.
            wvk = ppool.tile([128, KT * 128], f16)
            wqq = ppool.tile([128, KT * 128], f16)
            cst = ppool.tile([128, 192], f16)
            bvk = ppool.tile([128, 1], f32)
            bqq = ppool.tile([128, 1], f32)
            nc.sync.dma_start(out=wvk[:], in_=wvk_d[:])
            nc.gpsimd.dma_start(out=wqq[:], in_=wqq_d[:])
            nc.scalar.dma_start(out=cst[:], in_=cst_d[:])
            nc.scalar.dma_start(out=bvk[:], in_=bvk_d[:])
            nc.scalar.dma_start(out=bqq[:], in_=bqq_d[:])
            tri = cst[:, 0:128]
            ident = cst[0:64, 128:192]

            xts = [
                xpool.tile([128, T], f16, tag=f"x{k}", name=f"x{k}")
                for k in range(KT)
            ]
            nc.sync.dma_start(out=xts[0][:, 0:1024], in_=xt_d[0, :, 0:1024])
            nc.gpsimd.dma_start(out=xts[0][:, 1024:T], in_=xt_d[0, :, 1024:T])
            dma_engs = [nc.scalar, nc.sync, nc.gpsimd]
            for k in range(1, KT):
                dma_engs[(k - 1) % 3].dma_start(out=xts[k][:], in_=xt_d[k])

            # persistent activations
            vk = ppool.tile([128, T], f16, tag="vk")   # V.T rows 0:64, K.T rows 64:128
            qq = ppool.tile([128, T], f16, tag="qq")   # Q.T in both halves
            v1 = ppool.tile([128, NS * 65], f16, tag="v1")  # [V_i | 1] stationaries
            osb = ppool.tile([65, T], f32, tag="osb")

            nc.gpsimd.memset(v1[:], 1.0)

            # ---------------- projections ----------------
            with tc.tile_pool(name="pproj", bufs=1, space="PSUM") as pproj:
                psA = pproj.tile([128, T], f32, tag="psA")
                psB = pproj.tile([128, T], f32, tag="psB")
                for k in range(KT):
                    for ps, w in ((psA, wvk), (psB, wqq)):
                        for c in range(T // PCHUNK):
                            sl = slice(PCHUNK * c, PCHUNK * (c + 1))
                            nc.tensor.matmul(
                                ps[:, sl],
                                w[:, 128 * k:128 * (k + 1)],
                                xts[k][:, sl],
                                start=(k == 0), stop=(k == KT - 1),
                            )
                # drain with exact bias add, pipelined per 512-chunk
                for c in range(NCHUNK):
                    sl = slice(512 * c, 512 * (c + 1))
                    nc.vector.tensor_scalar_add(vk[:, sl], psA[:, sl], bvk[:])
                    nc.vector.tensor_scalar_add(qq[:, sl], psB[:, sl], bqq[:])

            # ---------------- V transposes ----------------
            with tc.tile_pool(name="pv", bufs=2, space="PSUM") as pv:
                for i in range(NS):
                    vt = pv.tile([128, 64], f16, tag="vt")
                    nc.tensor.transpose(
                        vt[:], vk[0:64, 128 * i:128 * (i + 1)], ident[:]
                    )
                    nc.vector.tensor_copy(v1[:, 65 * i:65 * i + 64], vt[:])

            # ---------------- attention ----------------
            # Per s-tile i (ts = 128*i), diagonal-trimmed:
            #   scores: S.T[:, ts:2048] via <=512-col matmuls into two
            #           rotating [128,1024] psum tiles (one per 1024-block)
            #   exp:    one ScalarE activation per 1024-block (f32 -> fp16,
            #           scale fused), tri mask on the diagonal 128-block
            #   PV:     po[j][:, max(512j,ts):] += [V_i|1]^T @ et for
            #           j = i//4..3  (sub-diagonal columns skipped)
            with tc.tile_pool(name="po", bufs=1, space="PSUM") as po, \
                 tc.tile_pool(name="pst", bufs=2, space="PSUM") as pst, \
                 tc.tile_pool(name="et", bufs=3) as etpool:

                ops = [
                    po.tile([65, 512], f32, tag=f"o{j}", name=f"o{j}")
                    for j in range(NCHUNK)
                ]

                ets = {}

                def scores_exp(i):
                    ts = 128 * i
                    et = etpool.tile([128, T], f16, tag="et")
                    ets[i] = et
                    for tb in range(ts // 1024, 2):
                        lo = max(ts, 1024 * tb)
                        hi = 1024 * (tb + 1)
                        st = pst.tile([128, 1024], f32, tag="st")
                        t0 = lo
                        while t0 < hi:
                            t1 = min(hi, (t0 // 512 + 1) * 512)
                            nc.tensor.matmul(
                                st[:, t0 - 1024 * tb:t1 - 1024 * tb],
                                vk[64:128, ts:ts + 128],
                                qq[64:128, t0:t1],
                                start=True, stop=True,
                            )
                            t0 = t1
                        nc.scalar.activation(
                            et[:, lo:hi], st[:, lo - 1024 * tb:1024],
                            EXP, scale=SCALE,
                        )
                        if lo == ts:  # diagonal block is in this piece
                            nc.vector.tensor_mul(
                                et[:, ts:ts + 128], et[:, ts:ts + 128], tri[:]
                            )

                def pv_acc(i):
                    ts = 128 * i
                    jmin = i // 4
                    et = ets.pop(i)
                    for j in range(jmin, NCHUNK):
                        c0 = max(512 * j, ts)
                        nc.tensor.matmul(
                            ops[j][:, c0 - 512 * j:],
                            v1[:, 65 * i:65 * i + 65],
                            et[:, c0:512 * (j + 1)],
                            start=(i == 0), stop=(i == 4 * j + 3),
                        )
                    for j in range(jmin, NCHUNK):
                        if i == 4 * j + 3:
                            sl = slice(512 * j, 512 * (j + 1))
                            nc.vector.tensor_copy(osb[:, sl], ops[j][:])
                            nc.sync.dma_start(out=out_d[:, sl], in_=osb[:, sl])

                scores_exp(0)
                for i in range(NS):
                    if i + 1 < NS:
                        scores_exp(i + 1)
                    pv_acc(i)

    nc.compile()
    return nc


def _get_nc():
    if "nc" not in _CACHE:
        _CACHE["nc"] = _build()
    return _CACHE["nc"]


def make_in_maps(X, Wq, bq, Wk, bk, Wv, bv):
    X = np.asarray(X, dtype=np.float32)
    Wq = np.asarray(Wq, dtype=np.float32)
    Wk = np.asarray(Wk, dtype=np.float32)
    Wv = np.asarray(Wv, dtype=np.float32)
    bq = np.asarray(bq, dtype=np.float32)
    bk = np.asarray(bk, dtype=np.float32)
    bv = np.asarray(bv, dtype=np.float32)

    wvk = np.ascontiguousarray(
        np.concatenate([Wv, Wk], axis=1).reshape(KT, 128, 128)
        .transpose(1, 0, 2).reshape(128, KT * 128)
    ).astype(np.float16)
    wqq = np.ascontiguousarray(
        np.concatenate([Wq, Wq], axis=1).reshape(KT, 128, 128)
        .transpose(1, 0, 2).reshape(128, KT * 128)
    ).astype(np.float16)
    bvk = np.concatenate([bv, bk]).reshape(128, 1).astype(np.float32)
    bqq = np.concatenate([bq, bq]).reshape(128, 1).astype(np.float32)

    in_maps = []
    for b in range(B):
        xt = (
            np.ascontiguousarray(X[b].T)
            .astype(np.float16)
            .reshape(KT, 128, T)
        )
        in_maps.append(
            {"xt": xt, "wvk": wvk, "wqq": wqq, "bvk": bvk, "bqq": bqq}
        )
    return in_maps


def kernel(X, Wq, bq, Wk, bk, Wv, bv):
    from concourse.bass_utils import run_bass_kernel_spmd

    nc = _get_nc()
    in_maps = make_in_maps(X, Wq, bq, Wk, bk, Wv, bv)
    res = run_bass_kernel_spmd(nc, in_maps, list(range(B)))

    out = np.empty((B, T, DK), dtype=np.float32)
    for b in range(B):
        r = res.results[b]["out"]
        out[b] = (r[:64] / r[64:65]).T
    return out


# revision 12
# speedup vs baseline: 1.2701x; 1.0072x over previous
"""Single-head causal self-attention on 8 NeuronCores (data-parallel over batch).

Reference computation (per batch element b):
    Q = X @ Wq + bq; K = X @ Wk + bk; V = X @ Wv + bv        # [T, DK]
    S = Q @ K.T / sqrt(DK)  (causal masked)
    out = softmax(S) @ V                                      # [T, DK]

Device strategy (one batch element per core), fp16 compute / fp32 accumulate:
  - Host passes X.T [C, T] in fp16 so every DMA row is contiguous and
    HBM+tunnel traffic is halved vs fp32.
  - DMA order puts the stationaries (wvk, wqq) and the first X tile at the
    head of the three DMA queues so the PE can start within ~5us of the
    queues opening instead of waiting behind 1.5MB of X.
  - Two full-width projection passes with packed fp16 stationaries:
      pass A: [Wv | Wk] -> psum rows 0:64 = V.T, rows 64:128 = K.T
      pass B: [Wq | Wq] -> Q.T duplicated in both partition halves
    Biases are added exactly during the PSUM->SBUF drain (tensor_scalar_add
    with a per-partition fp32 vector, fp16 output).
  - V.T is PE-transposed into natural [s, dk] tiles with a ones column
    appended; the ones column makes the output matmul also produce the
    softmax denominator l (row 64 of the output).
  - Scores are computed transposed: S.T[s, t] = K.T^T @ Q.T, so softmax's
    exp (ScalarE, scale=1/8 fused) and the P@V contraction (over s = the
    partition dim) need no large transposes.
  - Causality: score/exp/PV work is trimmed to start exactly at the
    diagonal column ts = 128*i (no sub-diagonal columns, no memsets);
    only the diagonal 128-block needs a triangular mask multiply.
  - Tensor-queue order runs one s-tile of score lookahead (S(i+1) issued
    before PV(i)) so the PE array does not stall waiting for exp(i).
  - Device output per core: [65, T] = rows 0:64 unnormalized O.T, row 64 l.
    Host computes (O_unnorm / l).T.
"""

import sys

sys.path.insert(0, "/opt/trn_rl_repo")

import numpy as np

B, T, C, DK = 8, 2048, 1024, 64
KT = C // 128          # 8 k-tiles in the contraction over C
NS = T // 128          # 16 s-tiles (key blocks)
NCHUNK = T // 512      # 4 output chunks of 512
SCALE = 1.0 / np.sqrt(DK)
PCHUNK = 512           # projection matmul column width

_CACHE = {}


def _build():
    from concourse import bass, bacc, tile

    mybir = bass.mybir
    f32 = mybir.dt.float32
    f16 = mybir.dt.float16

    nc = bacc.Bacc(
        "TRN2", target_bir_lowering=False, debug=False, num_devices=B
    )

    bf16 = mybir.dt.bfloat16

    xt_d = nc.dram_tensor("xt", [KT, 128, T], f16, kind="ExternalInput")
    wvk_d = nc.dram_tensor("wvk", [128, KT * 128], f16, kind="ExternalInput")
    wqq_d = nc.dram_tensor("wqq", [128, KT * 128], f16, kind="ExternalInput")
    bvk_d = nc.dram_tensor("bvk", [128, 1], f32, kind="ExternalInput")
    bqq_d = nc.dram_tensor("bqq", [128, 1], f32, kind="ExternalInput")
    # bf16 output (fp16 would overflow: l can reach ~3e5); host divides in f32
    out_d = nc.dram_tensor("out", [65, T], bf16, kind="ExternalOutput")

    # one packed const block: cols 0:128 tri-mask, 128:192 ident (rows 0:64)
    cst_np = np.zeros((128, 192), dtype=np.float16)
    cst_np[:, 0:128] = np.triu(np.ones((128, 128), dtype=np.float16))
    cst_np[0:64, 128:192] = np.eye(64, dtype=np.float16)
    cst_d = nc.inline_tensor(cst_np, "cst")

    EXP = mybir.ActivationFunctionType.Exp

    with tile.TileContext(nc) as tc:
        with tc.tile_pool(name="persist", bufs=1) as ppool, \
             tc.tile_pool(name="x", bufs=1) as xpool:

            # stationaries at the head of the queues, then x tiles
            # round-robin; x0 is split across sync+gpsimd so the first
            # projection matmul can start as early as possible.
            wvk = ppool.tile([128, KT * 128], f16)
            wqq = ppool.tile([128, KT * 128], f16)
            cst = ppool.tile([128, 192], f16)
            bvk = ppool.tile([128, 1], f32)
            bqq = ppool.tile([128, 1], f32)
            nc.sync.dma_start(out=wvk[:], in_=wvk_d[:])
            nc.gpsimd.dma_start(out=wqq[:], in_=wqq_d[:])
            nc.scalar.dma_start(out=cst[:], in_=cst_d[:])
            nc.scalar.dma_start(out=bvk[:], in_=bvk_d[:])
            nc.scalar.dma_start(out=bqq[:], in_=bqq_d[:])
            tri = cst[:, 0:128]
            ident = cst[0:64, 128:192]

            # each x tile is DMA'd in two halves, round-robin across the
            # three DMA queues, so tiles arrive in k order at aggregate
            # bandwidth and the PE never waits for the next k tile
            xts = [
                xpool.tile([128, T], f16, tag=f"x{k}", name=f"x{k}")
                for k in range(KT)
            ]
            dma_engs = [nc.sync, nc.gpsimd, nc.scalar]
            qi = 0
            for k in range(KT):
                for h in range(2):
                    sl = slice(1024 * h, 1024 * (h + 1))
                    dma_engs[qi % 3].dma_start(
                        out=xts[k][:, sl], in_=xt_d[k, :, sl]
                    )
                    qi += 1

            # persistent activations
            vk = ppool.tile([128, T], f16, tag="vk")   # V.T rows 0:64, K.T rows 64:128
            qq = ppool.tile([128, T], f16, tag="qq")   # Q.T in both halves
            v1 = ppool.tile([128, NS * 65], f16, tag="v1")  # [V_i | 1] stationaries
            osb = ppool.tile([65, T], bf16, tag="osb")

            nc.gpsimd.memset(v1[:], 1.0)

            # ---------------- projections ----------------
            # per-512-chunk psum tiles (8 tiles = all 8 banks) so each
            # chunk's drain waits only on its own 8-matmul accumulation
            # group, not on the whole pass
            with tc.tile_pool(name="pproj", bufs=1, space="PSUM") as pproj:
                psA = [
                    pproj.tile([128, 512], f32, tag=f"psA{c}", name=f"psA{c}")
                    for c in range(NCHUNK)
                ]
                psB = [
                    pproj.tile([128, 512], f32, tag=f"psB{c}", name=f"psB{c}")
                    for c in range(NCHUNK)
                ]
                for k in range(KT):
                    for ps, w in ((psA, wvk), (psB, wqq)):
                        for c in range(NCHUNK):
                            sl = slice(512 * c, 512 * (c + 1))
                            nc.tensor.matmul(
                                ps[c][:],
                                w[:, 128 * k:128 * (k + 1)],
                                xts[k][:, sl],
                                start=(k == 0), stop=(k == KT - 1),
                            )
                # drain with exact bias add, pipelined per 512-chunk
                for c in range(NCHUNK):
                    sl = slice(512 * c, 512 * (c + 1))
                    nc.vector.tensor_scalar_add(vk[:, sl], psA[c][:], bvk[:])
                    nc.vector.tensor_scalar_add(qq[:, sl], psB[c][:], bqq[:])

            # ---------------- V transposes ----------------
            with tc.tile_pool(name="pv", bufs=2, space="PSUM") as pv:
                for i in range(NS):
                    vt = pv.tile([128, 64], f16, tag="vt")
                    nc.tensor.transpose(
                        vt[:], vk[0:64, 128 * i:128 * (i + 1)], ident[:]
                    )
                    nc.vector.tensor_copy(v1[:, 65 * i:65 * i + 64], vt[:])

            # ---------------- attention ----------------
            # Per s-tile i (ts = 128*i), diagonal-trimmed:
            #   scores: S.T[:, ts:2048] via <=512-col matmuls into two
            #           rotating [128,1024] psum tiles (one per 1024-block)
            #   exp:    one ScalarE activation per 1024-block (f32 -> fp16,
            #           scale fused), tri mask on the diagonal 128-block
            #   PV:     po[j][:, max(512j,ts):] += [V_i|1]^T @ et for
            #           j = i//4..3  (sub-diagonal columns skipped)
            with tc.tile_pool(name="po", bufs=1, space="PSUM") as po, \
                 tc.tile_pool(name="pst", bufs=2, space="PSUM") as pst, \
                 tc.tile_pool(name="et", bufs=3) as etpool:

                ops = [
                    po.tile([65, 512], f32, tag=f"o{j}", name=f"o{j}")
                    for j in range(NCHUNK)
                ]

                ets = {}

                def scores_exp(i):
                    ts = 128 * i
                    et = etpool.tile([128, T], f16, tag="et")
                    ets[i] = et
                    for tb in range(ts // 1024, 2):
                        lo = max(ts, 1024 * tb)
                        hi = 1024 * (tb + 1)
                        st = pst.tile([128, 1024], f32, tag="st")
                        t0 = lo
                        while t0 < hi:
                            t1 = min(hi, (t0 // 512 + 1) * 512)
                            nc.tensor.matmul(
                                st[:, t0 - 1024 * tb:t1 - 1024 * tb],
                                vk[64:128, ts:ts + 128],
                                qq[64:128, t0:t1],
                                start=True, stop=True,
                            )
                            t0 = t1
                        nc.scalar.activation(
                            et[:, lo:hi], st[:, lo - 1024 * tb:1024],
                            EXP, scale=SCALE,
                        )
                        if lo == ts:  # diagonal block is in this piece
                            nc.vector.tensor_mul(
                                et[:, ts:ts + 128], et[:, ts:ts + 128], tri[:]
                            )

                def pv_acc(i):
                    ts = 128 * i
                    jmin = i // 4
                    et = ets.pop(i)
                    for j in range(jmin, NCHUNK):
                        c0 = max(512 * j, ts)
                        nc.tensor.matmul(
                            ops[j][:, c0 - 512 * j:],
                            v1[:, 65 * i:65 * i + 65],
                            et[:, c0:512 * (j + 1)],
                            start=(i == 0), stop=(i == 4 * j + 3),
                        )
                    for j in range(jmin, NCHUNK):
                        if i == 4 * j + 3:
                            sl = slice(512 * j, 512 * (j + 1))
                            nc.vector.tensor_copy(osb[:, sl], ops[j][:])
                            # split the out DMA across the three queues so
                            # the final chunk's writeback is not serial on
                            # one queue at tail time
                            for h, eng in enumerate(
                                (nc.sync, nc.gpsimd, nc.scalar)
                            ):
                                c0 = 512 * j + 171 * h
                                c1 = 512 * j + (171 * (h + 1) if h < 2 else 512)
                                eng.dma_start(
                                    out=out_d[:, c0:c1], in_=osb[:, c0:c1]
                                )

                scores_exp(0)
                for i in range(NS):
                    if i + 1 < NS:
                        scores_exp(i + 1)
                    pv_acc(i)

    nc.compile()
    return nc


def _get_nc():
    if "nc" not in _CACHE:
        _CACHE["nc"] = _build()
    return _CACHE["nc"]


def make_in_maps(X, Wq, bq, Wk, bk, Wv, bv):
    X = np.asarray(X, dtype=np.float32)
    Wq = np.asarray(Wq, dtype=np.float32)
    Wk = np.asarray(Wk, dtype=np.float32)
    Wv = np.asarray(Wv, dtype=np.float32)
    bq = np.asarray(bq, dtype=np.float32)
    bk = np.asarray(bk, dtype=np.float32)
    bv = np.asarray(bv, dtype=np.float32)

    wvk = np.ascontiguousarray(
        np.concatenate([Wv, Wk], axis=1).reshape(KT, 128, 128)
        .transpose(1, 0, 2).reshape(128, KT * 128)
    ).astype(np.float16)
    wqq = np.ascontiguousarray(
        np.concatenate([Wq, Wq], axis=1).reshape(KT, 128, 128)
        .transpose(1, 0, 2).reshape(128, KT * 128)
    ).astype(np.float16)
    bvk = np.concatenate([bv, bk]).reshape(128, 1).astype(np.float32)
    bqq = np.concatenate([bq, bq]).reshape(128, 1).astype(np.float32)

    in_maps = []
    for b in range(B):
        xt = (
            np.ascontiguousarray(X[b].T)
            .astype(np.float16)
            .reshape(KT, 128, T)
        )
        in_maps.append(
            {"xt": xt, "wvk": wvk, "wqq": wqq, "bvk": bvk, "bqq": bqq}
        )
    return in_maps


def kernel(X, Wq, bq, Wk, bk, Wv, bv):
    from concourse.bass_utils import run_bass_kernel_spmd

    nc = _get_nc()
    in_maps = make_in_maps(X, Wq, bq, Wk, bk, Wv, bv)
    res = run_bass_kernel_spmd(nc, in_maps, list(range(B)))

    out = np.empty((B, T, DK), dtype=np.float32)
    for b in range(B):
        r = np.asarray(res.results[b]["out"], dtype=np.float32)
        out[b] = (r[:64] / r[64:65]).T
    return out


# revision 16
# speedup vs baseline: 1.3515x; 1.0641x over previous
"""Single-head causal self-attention on 8 NeuronCores (data-parallel over batch).

Reference computation (per batch element b):
    Q = X @ Wq + bq; K = X @ Wk + bk; V = X @ Wv + bv        # [T, DK]
    S = Q @ K.T / sqrt(DK)  (causal masked)
    out = softmax(S) @ V                                      # [T, DK]

Device strategy (one batch element per core), fp16 compute / fp32 accumulate:
  - Host passes X.T [C, T] in fp16: contiguous DMA rows, half the
    HBM/tunnel traffic of fp32, and fp16 stationaries load via LDWEIGHTS
    that overlap the previous matmul (fp32 self-loads serialize).
  - X tiles are DMA'd in 0.25MB halves round-robin across the three DMA
    queues, stationaries at the queue heads, so the PE starts ~13us in
    and k-tiles keep arriving just ahead of the PE.
  - Two full-width projection passes with packed fp16 stationaries:
      pass A: [Wv | Wk] -> psum rows 0:64 = V.T, rows 64:128 = K.T
      pass B: [Wq | Wq] -> Q.T duplicated in both partition halves
    accumulated in eight per-512-chunk psum tiles (all 8 banks); chunk
    order is interleaved A0,B0,A1,B1,... so early chunks' accumulation
    groups close first and their drains overlap the projection tail.
  - Drains (exact bias add, fp16 out) are split: V/K chunks on VectorE,
    Q chunks on GpSimd, so both run in parallel right after each chunk's
    group closes; V transposes then only wait for their own chunk.
  - V.T is PE-transposed into natural [s, dk] tiles with a ones column
    appended; the ones column makes the output matmul also produce the
    softmax denominator l (row 64 of the output).
  - Scores are computed transposed: S.T[s, t] = K.T^T @ Q.T, one matmul
    per 1024-block (non-accumulating matmuls may span both psum banks of
    the block's tile), trimmed to start exactly at the diagonal column
    ts = 128*i.  exp (ScalarE, f32 psum -> fp16, scale=1/8 fused) per
    block; triangular mask multiply only on the diagonal 128-block.
  - PV: po[j][:, max(512j,ts):] += [V_i|1]^T @ exp-piece for j=i//4..3
    (sub-diagonal columns skipped entirely - no memsets).  Tensor-queue
    order runs one s-tile of score lookahead (S(i+1) before PV(i)) so
    the PE does not stall waiting for exp(i).
  - Device output per core: [65, T] bf16 = rows 0:64 unnormalized O.T,
    row 64 the denominator l (fp16 would overflow: l reaches ~3e5).
    Each chunk's writeback is split across the three DMA queues.  Host
    computes (O_unnorm / l).T in fp32.
"""

import sys

sys.path.insert(0, "/opt/trn_rl_repo")

import numpy as np

B, T, C, DK = 8, 2048, 1024, 64
KT = C // 128          # 8 k-tiles in the contraction over C
NS = T // 128          # 16 s-tiles (key blocks)
NCHUNK = T // 512      # 4 output chunks of 512
SCALE = 1.0 / np.sqrt(DK)

_CACHE = {}


def _build():
    from concourse import bass, bacc, tile

    mybir = bass.mybir
    f32 = mybir.dt.float32
    f16 = mybir.dt.float16
    bf16 = mybir.dt.bfloat16

    nc = bacc.Bacc(
        "TRN2", target_bir_lowering=False, debug=False, num_devices=B
    )

    xt_d = nc.dram_tensor("xt", [KT, 128, T], f16, kind="ExternalInput")
    wvk_d = nc.dram_tensor("wvk", [128, KT * 128], f16, kind="ExternalInput")
    wqq_d = nc.dram_tensor("wqq", [128, KT * 128], f16, kind="ExternalInput")
    bvk_d = nc.dram_tensor("bvk", [128, 1], f32, kind="ExternalInput")
    bqq_d = nc.dram_tensor("bqq", [128, 1], f32, kind="ExternalInput")
    out_d = nc.dram_tensor("out", [65, T], bf16, kind="ExternalOutput")

    # one packed const block: cols 0:128 tri-mask, 128:192 ident (rows 0:64)
    cst_np = np.zeros((128, 192), dtype=np.float16)
    cst_np[:, 0:128] = np.triu(np.ones((128, 128), dtype=np.float16))
    cst_np[0:64, 128:192] = np.eye(64, dtype=np.float16)
    cst_d = nc.inline_tensor(cst_np, "cst")

    EXP = mybir.ActivationFunctionType.Exp

    with tile.TileContext(nc) as tc:
        with tc.tile_pool(name="persist", bufs=1) as ppool, \
             tc.tile_pool(name="x", bufs=1) as xpool:

            wvk = ppool.tile([128, KT * 128], f16)
            wqq = ppool.tile([128, KT * 128], f16)
            cst = ppool.tile([128, 192], f16)
            bvk = ppool.tile([128, 1], f32)
            bqq = ppool.tile([128, 1], f32)
            nc.sync.dma_start(out=wvk[:], in_=wvk_d[:])
            nc.gpsimd.dma_start(out=wqq[:], in_=wqq_d[:])
            nc.scalar.dma_start(out=cst[:], in_=cst_d[:])
            nc.scalar.dma_start(out=bvk[:], in_=bvk_d[:])
            nc.scalar.dma_start(out=bqq[:], in_=bqq_d[:])
            tri = cst[:, 0:128]
            ident = cst[0:64, 128:192]

            xts = [
                xpool.tile([128, T], f16, tag=f"x{k}", name=f"x{k}")
                for k in range(KT)
            ]
            dma_engs = [nc.sync, nc.gpsimd, nc.scalar]
            for qi in range(2 * KT):
                k, h = qi // 2, qi % 2
                sl = slice(1024 * h, 1024 * (h + 1))
                # k7h1 goes to scalar (not sync) to balance queue bytes
                eng = dma_engs[2 if qi == 15 else qi % 3]
                eng.dma_start(out=xts[k][:, sl], in_=xt_d[k, :, sl])

            # persistent activations; vk/qq are per-chunk tiles so each
            # consumer waits only on its own chunk's drain
            vks = [
                ppool.tile([128, 512], f16, tag=f"vk{c}", name=f"vk{c}")
                for c in range(NCHUNK)
            ]
            qqs = [
                ppool.tile([128, 1024], f16, tag=f"qq{tb}", name=f"qq{tb}")
                for tb in range(2)
            ]
            v1 = ppool.tile([128, NS * 65], f16, tag="v1")  # [V_i | 1] stationaries
            osb = ppool.tile([65, T], bf16, tag="osb")

            nc.gpsimd.memset(v1[:], 1.0)

            # ---------------- projections ----------------
            with tc.tile_pool(name="pproj", bufs=1, space="PSUM") as pproj:
                psA = [
                    pproj.tile([128, 512], f32, tag=f"psA{c}", name=f"psA{c}")
                    for c in range(NCHUNK)
                ]
                psB = [
                    pproj.tile([128, 512], f32, tag=f"psB{c}", name=f"psB{c}")
                    for c in range(NCHUNK)
                ]
                for k in range(KT):
                    for c in range(NCHUNK):
                        sl = slice(512 * c, 512 * (c + 1))
                        for ps, w in ((psA, wvk), (psB, wqq)):
                            nc.tensor.matmul(
                                ps[c][:],
                                w[:, 128 * k:128 * (k + 1)],
                                xts[k][:, sl],
                                start=(k == 0), stop=(k == KT - 1),
                            )
                # drains: V/K on VectorE, Q on ScalarE (parallel; GpSimd
                # cannot touch PSUM).  ScalarE's Identity activation with a
                # per-partition AP bias is an exact bias add.
                IDENT_FN = mybir.ActivationFunctionType.Identity
                for c in range(NCHUNK):
                    nc.vector.tensor_scalar_add(vks[c][:], psA[c][:], bvk[:])
                    nc.scalar.activation(
                        qqs[c // 2][:, 512 * (c % 2):512 * (c % 2) + 512],
                        psB[c][:], IDENT_FN, bias=bqq[:],
                    )

            # ---------------- V transposes ----------------
            with tc.tile_pool(name="pv", bufs=2, space="PSUM") as pv:
                for i in range(NS):
                    vt = pv.tile([128, 64], f16, tag="vt")
                    nc.tensor.transpose(
                        vt[:], vks[i // 4][0:64, 128 * (i % 4):128 * (i % 4) + 128],
                        ident[:],
                    )
                    nc.vector.tensor_copy(v1[:, 65 * i:65 * i + 64], vt[:])

            # ---------------- attention ----------------
            with tc.tile_pool(name="po", bufs=1, space="PSUM") as po, \
                 tc.tile_pool(name="pst", bufs=2, space="PSUM") as pst, \
                 tc.tile_pool(name="et", bufs=3) as etpool:

                ops = [
                    po.tile([65, 512], f32, tag=f"o{j}", name=f"o{j}")
                    for j in range(NCHUNK)
                ]

                ets = {}

                def scores_exp(i):
                    ts = 128 * i
                    et = etpool.tile([128, T], f16, tag="et")
                    ets[i] = et
                    for tb in range(ts // 1024, 2):
                        lo = max(ts, 1024 * tb)
                        hi = 1024 * (tb + 1)
                        st = pst.tile([128, 1024], f32, tag="st")
                        t0 = lo
                        while t0 < hi:  # matmul out must stay in one bank
                            t1 = min(hi, (t0 // 512 + 1) * 512)
                            nc.tensor.matmul(
                                st[:, t0 - 1024 * tb:t1 - 1024 * tb],
                                vks[i // 4][
                                    64:128, 128 * (i % 4):128 * (i % 4) + 128
                                ],
                                qqs[tb][64:128, t0 - 1024 * tb:t1 - 1024 * tb],
                                start=True, stop=True,
                            )
                            t0 = t1
                        nc.scalar.activation(
                            et[:, lo:hi], st[:, lo - 1024 * tb:1024],
                            EXP, scale=SCALE,
                        )
                        if lo == ts:  # diagonal block is in this piece
                            nc.vector.tensor_mul(
                                et[:, ts:ts + 128], et[:, ts:ts + 128], tri[:]
                            )

                def pv_acc(i):
                    ts = 128 * i
                    jmin = i // 4
                    et = ets.pop(i)
                    for j in range(jmin, NCHUNK):
                        c0 = max(512 * j, ts)
                        nc.tensor.matmul(
                            ops[j][:, c0 - 512 * j:],
                            v1[:, 65 * i:65 * i + 65],
                            et[:, c0:512 * (j + 1)],
                            start=(i == 0), stop=(i == 4 * j + 3),
                        )
                    for j in range(jmin, NCHUNK):
                        if i == 4 * j + 3:
                            sl = slice(512 * j, 512 * (j + 1))
                            nc.vector.tensor_copy(osb[:, sl], ops[j][:])
                            for h, eng in enumerate(
                                (nc.sync, nc.gpsimd, nc.scalar)
                            ):
                                c0 = 512 * j + 171 * h
                                c1 = 512 * j + (171 * (h + 1) if h < 2 else 512)
                                eng.dma_start(
                                    out=out_d[:, c0:c1], in_=osb[:, c0:c1]
                                )

                scores_exp(0)
                for i in range(NS):
                    if i + 1 < NS:
                        scores_exp(i + 1)
                    pv_acc(i)

    nc.compile()
    return nc


def _get_nc():
    if "nc" not in _CACHE:
        _CACHE["nc"] = _build()
    return _CACHE["nc"]


def make_in_maps(X, Wq, bq, Wk, bk, Wv, bv):
    X = np.asarray(X, dtype=np.float32)
    Wq = np.asarray(Wq, dtype=np.float32)
    Wk = np.asarray(Wk, dtype=np.float32)
    Wv = np.asarray(Wv, dtype=np.float32)
    bq = np.asarray(bq, dtype=np.float32)
    bk = np.asarray(bk, dtype=np.float32)
    bv = np.asarray(bv, dtype=np.float32)

    wvk = np.ascontiguousarray(
        np.concatenate([Wv, Wk], axis=1).reshape(KT, 128, 128)
        .transpose(1, 0, 2).reshape(128, KT * 128)
    ).astype(np.float16)
    wqq = np.ascontiguousarray(
        np.concatenate([Wq, Wq], axis=1).reshape(KT, 128, 128)
        .transpose(1, 0, 2).reshape(128, KT * 128)
    ).astype(np.float16)
    bvk = np.concatenate([bv, bk]).reshape(128, 1).astype(np.float32)
    bqq = np.concatenate([bq, bq]).reshape(128, 1).astype(np.float32)

    in_maps = []
    for b in range(B):
        xt = (
            np.ascontiguousarray(X[b].T)
            .astype(np.float16)
            .reshape(KT, 128, T)
        )
        in_maps.append(
            {"xt": xt, "wvk": wvk, "wqq": wqq, "bvk": bvk, "bqq": bqq}
        )
    return in_maps


def kernel(X, Wq, bq, Wk, bk, Wv, bv):
    from concourse.bass_utils import run_bass_kernel_spmd

    nc = _get_nc()
    in_maps = make_in_maps(X, Wq, bq, Wk, bk, Wv, bv)
    res = run_bass_kernel_spmd(nc, in_maps, list(range(B)))

    out = np.empty((B, T, DK), dtype=np.float32)
    for b in range(B):
        r = np.asarray(res.results[b]["out"], dtype=np.float32)
        out[b] = (r[:64] / r[64:65]).T
    return out
